# revision 63
# baseline (speedup 1.0000x reference)
"""Trainium2 Bass kernel for batched tanh-query attention.

Per-batch computation (B=8, one batch per NeuronCore, pure data parallel):
    q = tanh(out_state)            [Q, H]    Q=K=2048, H=128
    S = q @ history.T              [Q, K]
    P = softmax(S, axis=K)
    attn = P @ history             [Q, H]

Flash-style in the transposed orientation S_T[k, q] (no transpose of P
needed for the second matmul). Queries processed in 4 quarters of 512
columns; within a quarter a software pipeline at PAIR granularity
(2 k-tiles = 1024 score columns) keeps the Activation engine (the
bottleneck: exp at 1 elem/lane/cycle) saturated:
    PE : MM1 pair p (2x512-col) | MM2 pair p-1 (2 accumulating matmuls)
    ACT: exp over st[128,1024] f32 PSUM -> ex bf16 SBUF
    DVE: binary tree of bf16 adds for the softmax denominator, epilogue
    Pool: f32->bf16 conversion of history chunks
The denominator column d[q] per 128-q tile comes from a tiny matmul
tfin_tile.T @ ones -> [128,1] PSUM, avoiding row-transposes.
PSUM budget (8 banks): st x2 (4) + acc x2 (2) + dcol (1) + tsp (1).
"""

import os
import sys

os.environ.setdefault("NEURON_RT_RESET_CORES", "1")
for _p in ("/opt/trn_rl_repo", "/opt/trn_rl_repo/concourse"):
    if _p not in sys.path:
        sys.path.insert(0, _p)

import numpy as np

N_CORES = 8
SEQ = 2048
H = 128
P = 128
T = SEQ // P          # 16 seq tiles
NQ = 4                # query quarters
QW = SEQ // NQ        # 512
QTPQ = QW // P        # 4 q-tiles per quarter
NPAIR = T // 2        # 8 kb-pairs per quarter

_CACHE = {}


def _build():
    from concourse import bacc, bass, masks, mybir, tile

    f32 = mybir.dt.float32
    bf16 = mybir.dt.bfloat16
    AF = mybir.ActivationFunctionType

    nc = bacc.Bacc("TRN2", target_bir_lowering=False, debug=False,
                   num_devices=N_CORES)
    os_d = nc.dram_tensor("out_state", (SEQ, H), f32, kind="ExternalInput")
    h_d = nc.dram_tensor("history", (SEQ, H), f32, kind="ExternalInput")
    a_d = nc.dram_tensor("attn", (SEQ, H), f32, kind="ExternalOutput")

    with tile.TileContext(nc) as tc:
        with (
            tc.tile_pool(name="const", bufs=1) as constp,
            tc.tile_pool(name="big", bufs=1) as bigp,
            tc.tile_pool(name="ex", bufs=3) as expool,
            tc.tile_pool(name="tree", bufs=2) as treep,
            tc.tile_pool(name="work", bufs=2) as workp,
            tc.tile_pool(name="rcp", bufs=4) as rcp,
            tc.tile_pool(name="psst", bufs=2, space=bass.MemorySpace.PSUM) as psst,
            tc.tile_pool(name="pacc", bufs=2, space=bass.MemorySpace.PSUM) as pacc,
            tc.tile_pool(name="psdc", bufs=1, space=bass.MemorySpace.PSUM) as psdc,
            tc.tile_pool(name="pstp", bufs=1, space=bass.MemorySpace.PSUM) as pstp,
        ):
            # ---- input DMAs first so transfers overlap const setup ----
            os_f = bigp.tile([P, T, H], f32, tag="osf")
            hn_f = bigp.tile([P, T, H], f32, tag="hnf")
            os_v = os_d[:].rearrange("(t p) h -> p t h", p=P)
            hn_v = h_d[:].rearrange("(t p) h -> p t h", p=P)
            a_v = a_d[:].rearrange("(t p) h -> p t h", p=P)
            # os chunks first on sync (so ALL tanh work lands inside the
            # initial DMA-wait window); hn chunk 0 in parallel from the
            # scalar queue (idle until its DMA lands anyway)
            nc.sync.dma_start(os_f[:, 0:4, :], os_v[:, 0:4, :])
            nc.scalar.dma_start(hn_f[:, 0:4, :], hn_v[:, 0:4, :])
            nc.sync.dma_start(os_f[:, 4:8, :], os_v[:, 4:8, :])
            nc.sync.dma_start(os_f[:, 8:16, :], os_v[:, 8:16, :])
            nc.sync.dma_start(hn_f[:, 4:8, :], hn_v[:, 4:8, :])
            nc.sync.dma_start(hn_f[:, 8:12, :], hn_v[:, 8:12, :])
            nc.sync.dma_start(hn_f[:, 12:16, :], hn_v[:, 12:16, :])

            id_bf = constp.tile([P, P], bf16, tag="idb")
            masks.make_identity(nc, id_bf[:])
            ones_bf = constp.tile([P, 1], bf16, tag="ones")
            nc.vector.memset(ones_bf[:], 1.0)

            # persistent bf16 operands
            hn = bigp.tile([P, T, P], bf16, tag="hn")    # [k_in, t, h] natural
            ht = bigp.tile([P, T, P], bf16, tag="ht")    # [h, t, k_in]
            qT = bigp.tile([P, T, P], bf16, tag="qT")    # [h, t, q_in]
            q_nat = bigp.tile([P, T, H], bf16, tag="qnat")
            ot_all = bigp.tile([P, T, H], f32, tag="ot")  # output staging

            # ---- prologue compute ----
            # history f32 -> bf16 on DVE (idle until the first tree add;
            # Pool's software CAST is ~3x slower than DVE here)
            nc.vector.tensor_copy(hn[:, 0:4, :], hn_f[:, 0:4, :])
            nc.scalar.activation(q_nat[:, 0:4, :], os_f[:, 0:4, :], AF.Tanh)
            # history converts are split between DVE (fast, used for the
            # earliest-needed pairs) and Pool (slow but otherwise idle)
            nc.gpsimd.tensor_copy(hn[:, 12:14, :], hn_f[:, 12:14, :])
            nc.gpsimd.tensor_copy(hn[:, 14:16, :], hn_f[:, 14:16, :])

            # PE warm-up: the tensor engine ramps its clock only after ~3us
            # of continuous work; burn the DMA-wait window with dummy
            # transposes so the first real matmuls run at full speed
            wrm = pstp.tile([P, 8, P], bf16, tag="tsp", name="warm")
            for i in range(32):
                nc.tensor.transpose(wrm[:, i % 8, :], id_bf[:], id_bf[:])
            wrm_keep = constp.tile([P, 1], bf16, tag="wk")
            nc.vector.tensor_copy(wrm_keep[:], wrm[:, 0, 0:1])

            # one batch of PE transposes through the single tsp PSUM bank,
            # then one DVE copy out per destination range
            def tp_batch(jobs):
                # jobs: list of (dst_tile, dst_t0, src_tile, src_t0, n)
                ntot = sum(j[4] for j in jobs)
                tsp = pstp.tile([P, 8, P], bf16, tag="tsp", name="tsp")
                s = 0
                for dst, dt0, src, st0, n in jobs:
                    for i in range(n):
                        nc.tensor.transpose(tsp[:, s + i, :],
                                            src[:, st0 + i, :], id_bf[:])
                    s += n
                s = 0
                for dst, dt0, src, st0, n in jobs:
                    nc.vector.tensor_copy(dst[:, dt0:dt0 + n, :],
                                          tsp[:, s:s + n, :])
                    s += n

            tp_batch([(qT, 0, q_nat, 0, 4), (ht, 0, hn, 0, 2)])
            tp_batch([(ht, 2, hn, 2, 2)])

            # ---- epilogue helper: one quarter's outputs ----
            # returns closures to be emitted at chosen pipeline slots
            def make_epilogue(q, acc, dsrcs):
                st8 = {}

                def pe_dcols():
                    # one-column accumulating matmuls into one PSUM tile,
                    # then a single reciprocal over all 4 columns
                    dc = psdc.tile([P, QTPQ], f32, tag="dc", name=f"dc{q}")
                    for t in range(QTPQ):
                        for i, src in enumerate(dsrcs):
                            nc.tensor.matmul(dc[:, t:t + 1],
                                             src[:, P * t:P * (t + 1)],
                                             ones_bf[:], start=(i == 0),
                                             stop=(i == len(dsrcs) - 1))
                    rc = rcp.tile([P, QTPQ], f32, tag="rc", name=f"rc{q}")
                    nc.vector.reciprocal(rc[:], dc[:])
                    st8["rc"] = rc

                def dve_copy():
                    aT = workp.tile([P, QW], bf16, tag="at", name=f"aT{q}")
                    if q == NQ - 1:
                        # split between DVE and the now-idle ACT engine so
                        # the tail's transposes start half a copy earlier
                        nc.vector.tensor_copy(aT[:, 0:QW // 2],
                                              acc[:, 0:QW // 2])
                        nc.scalar.activation(aT[:, QW // 2:],
                                             acc[:, QW // 2:], AF.Copy)
                    else:
                        nc.vector.tensor_copy(aT[:], acc[:])
                    st8["aT"] = aT

                def pe_transposes():
                    tsp = pstp.tile([P, 8, P], bf16, tag="tsp", name=f"ep{q}")
                    for t in range(QTPQ):
                        nc.tensor.transpose(tsp[:, t, :],
                                            st8["aT"][:, P * t:P * (t + 1)],
                                            id_bf[:])
                    st8["tsp"] = tsp

                def dve_muls():
                    for t in range(QTPQ):
                        if q == NQ - 1 and t >= 2:
                            # ACT is idle after the last exp; Copy-with-scale
                            # halves the tail's serial mul chain
                            nc.scalar.activation(
                                ot_all[:, QTPQ * q + t, :],
                                st8["tsp"][:, t, :], AF.Copy,
                                scale=st8["rc"][:, t:t + 1])
                        else:
                            nc.vector.tensor_scalar_mul(
                                ot_all[:, QTPQ * q + t, :],
                                st8["tsp"][:, t, :], st8["rc"][:, t:t + 1])

                def dma_out():
                    if q == NQ - 1:
                        # split across the sync and scalar DGE queues: the
                        # scalar queue sits right behind its own Copy-muls,
                        # so both halves dispatch and transfer in parallel
                        nc.sync.dma_start(a_v[:, QTPQ * q:QTPQ * q + 2, :],
                                          ot_all[:, QTPQ * q:QTPQ * q + 2, :])
                        nc.scalar.dma_start(
                            a_v[:, QTPQ * q + 2:QTPQ * (q + 1), :],
                            ot_all[:, QTPQ * q + 2:QTPQ * (q + 1), :])
                    else:
                        nc.sync.dma_start(a_v[:, QTPQ * q:QTPQ * (q + 1), :],
                                          ot_all[:, QTPQ * q:QTPQ * (q + 1), :])

                return pe_dcols, dve_copy, pe_transposes, dve_muls, dma_out

            # ---- main pipeline ----
            accs = [None] * NQ
            ex_tiles = [[None] * NPAIR for _ in range(NQ)]
            epi = [None] * NQ

            def emit_mm1(q, p):
                st = psst.tile([P, 2 * QW], f32, tag="st", name=f"st{q}{p}")
                rhs = qT[:, QTPQ * q:QTPQ * (q + 1), :]
                nc.tensor.matmul(st[:, 0:QW], ht[:, 2 * p, :], rhs,
                                 start=True, stop=True)
                nc.tensor.matmul(st[:, QW:], ht[:, 2 * p + 1, :], rhs,
                                 start=True, stop=True)
                return st

            def emit_exp(q, p, st):
                ex = expool.tile([P, 2 * QW], bf16, tag="ex", name=f"ex{q}{p}")
                if q == NQ - 1 and p == NPAIR - 1:
                    # split the very last exp so the final MM2 (and with it
                    # the whole output tail) starts half an exp earlier
                    nc.scalar.activation(ex[:, 0:QW], st[:, 0:QW], AF.Exp)
                    nc.scalar.activation(ex[:, QW:], st[:, QW:], AF.Exp)
                else:
                    nc.scalar.activation(ex[:], st[:], AF.Exp)
                ex_tiles[q][p] = ex

            def emit_mm2(q, p):
                if accs[q] is None:
                    accs[q] = pacc.tile([P, QW], f32, tag="acc",
                                        name=f"acc{q}")
                for kb in (2 * p, 2 * p + 1):
                    nc.tensor.matmul(
                        accs[q][:], hn[:, kb, :],
                        ex_tiles[q][p][:, QW * (kb % 2):QW * (kb % 2 + 1)],
                        start=(kb == 0), stop=(kb == T - 1))

            # denominator tree state per quarter
            l1s = [[None, None, None, None] for _ in range(NQ)]
            l2s = [[None, None] for _ in range(NQ)]
            tfins = [None] * NQ

            def emit_tree(q, p):
                # called at odd p: add pair (p-1, p); fold levels when ready
                i = p // 2
                t1 = treep.tile([P, 2 * QW], bf16, tag="l1", name=f"l1_{q}{i}")
                nc.vector.tensor_add(t1[:], ex_tiles[q][p - 1][:],
                                     ex_tiles[q][p][:])
                l1s[q][i] = t1
                if p in (3, 7):
                    j = p // 4
                    t2 = treep.tile([P, 2 * QW], bf16, tag="l2",
                                    name=f"l2_{q}{j}")
                    nc.vector.tensor_add(t2[:], l1s[q][2 * j][:],
                                         l1s[q][2 * j + 1][:])
                    l2s[q][j] = t2
                if p == 7:
                    t3 = treep.tile([P, 2 * QW], bf16, tag="l3", name=f"l3_{q}")
                    nc.vector.tensor_add(t3[:], l2s[q][0][:], l2s[q][1][:])
                    tf = treep.tile([P, QW], bf16, tag="tf", name=f"tf{q}")
                    nc.vector.tensor_add(tf[:], t3[:, 0:QW], t3[:, QW:])
                    tfins[q] = tf

            def emit_tree_last(q, p):
                # last quarter: running sum so only the final pair's fold
                # sits on the critical path after the last exp
                i = p // 2
                if p in (1, 3, 5):
                    t1 = treep.tile([P, 2 * QW], bf16, tag="l1",
                                    name=f"l1_{q}{i}")
                    nc.vector.tensor_add(t1[:], ex_tiles[q][p - 1][:],
                                         ex_tiles[q][p][:])
                    l1s[q][i] = t1
                if p == 3:
                    t2 = treep.tile([P, 2 * QW], bf16, tag="l2",
                                    name=f"l2_{q}0")
                    nc.vector.tensor_add(t2[:], l1s[q][0][:], l1s[q][1][:])
                    l2s[q][0] = t2
                if p == 5:
                    t2 = treep.tile([P, 2 * QW], bf16, tag="l2",
                                    name=f"l2_{q}1")
                    nc.vector.tensor_add(t2[:], l2s[q][0][:], l1s[q][2][:])
                    l2s[q][1] = t2
                if p == 6:
                    # fold in pair 6 and collapse to one 512-wide tile
                    t3 = treep.tile([P, 2 * QW], bf16, tag="l3", name=f"l3_{q}")
                    nc.vector.tensor_add(t3[:], l2s[q][1][:],
                                         ex_tiles[q][6][:])
                    tf = treep.tile([P, QW], bf16, tag="tf", name=f"tf{q}")
                    nc.vector.tensor_add(tf[:], t3[:, 0:QW], t3[:, QW:])
                    tfins[q] = tf

            def fold_last_pair(q):
                # fold the final pair's exp tile to 512 wide on DVE; the
                # epilogue dcols then accumulate tfin + this fold
                f7 = treep.tile([P, QW], bf16, tag="f7", name=f"f7_{q}")
                nc.vector.tensor_add(f7[:], ex_tiles[q][NPAIR - 1][:, 0:QW],
                                     ex_tiles[q][NPAIR - 1][:, QW:])
                return f7

            for q in range(NQ):
                last = q == NQ - 1
                # MM1 leads the exp stream by two pairs so each exp's input
                # is complete well before the previous exp retires; the
                # leading MM1 is emitted LAST in each slot so batch/epilogue
                # PE work fills the WAR wait on the st buffer
                sts = {0: emit_mm1(q, 0), 1: emit_mm1(q, 1)}
                for p in range(NPAIR):
                    emit_exp(q, p, sts.pop(p))
                    if p > 0:
                        emit_mm2(q, p - 1)
                    if last:
                        if p % 2 == 1 or p == 6:
                            emit_tree_last(q, p)
                    elif p % 2 == 1:
                        emit_tree(q, p)
                    if q == 0 and p in (2, 4, 6):
                        # tanh for later quarters, filling ACT stalls while
                        # quarter 0 waits on the history convert chain
                        j0 = 2 * p
                        nc.scalar.activation(q_nat[:, j0:j0 + 4, :],
                                             os_f[:, j0:j0 + 4, :], AF.Tanh)
                    if q == 0:
                        # DVE casts for the earliest-needed history pairs,
                        # then ht transposes one pair ahead of their consumer
                        if p <= 3:
                            j0 = 4 + 2 * p
                            nc.vector.tensor_copy(hn[:, j0:j0 + 2, :],
                                                  hn_f[:, j0:j0 + 2, :])
                        if p <= 5:
                            t0 = 2 * p + 4
                            tp_batch([(ht, t0, hn, t0, 2)])
                    if q >= 1 and epi[q - 1] is not None:
                        pe_dcols, dve_copy, pe_trans, dve_muls, dma_out = \
                            epi[q - 1]
                        if p == 0:
                            # slot 0 has no MM2, so the epilogue's PE burst
                            # (d-column matmuls) lands in spare PE time
                            # instead of displacing the leading MM1s
                            dve_copy()
                            pe_dcols()
                        elif p == 1:
                            pe_trans()
                        elif p == 2:
                            dve_muls()
                        elif p == 3:
                            dma_out()
                            epi[q - 1] = None
                    if p == 7 and q < NQ - 1:
                        tp_batch([(qT, QTPQ * (q + 1), q_nat,
                                   QTPQ * (q + 1), 4)])
                    if p + 2 < NPAIR:
                        sts[p + 2] = emit_mm1(q, p + 2)
                # close the quarter
                emit_mm2(q, NPAIR - 1)
                if last:
                    f7 = fold_last_pair(q)
                    epi[q] = make_epilogue(q, accs[q], [tfins[q], f7])
                else:
                    epi[q] = make_epilogue(q, accs[q], [tfins[q]])

            # final quarter epilogue, emitted tight
            pe_dcols, dve_copy, pe_trans, dve_muls, dma_out = epi[NQ - 1]
            dve_copy()
            pe_dcols()
            pe_trans()
            dve_muls()
            dma_out()

    nc.compile()
    return nc


def _get_nc():
    if "nc" not in _CACHE:
        _CACHE["nc"] = _build()
    return _CACHE["nc"]


def _run(out_state, history, trace=False):
    from concourse.bass_utils import run_bass_kernel_spmd

    nc = _get_nc()
    out_state = np.ascontiguousarray(out_state, dtype=np.float32)
    history = np.ascontiguousarray(history, dtype=np.float32)
    in_maps = [
        {"out_state": out_state[b], "history": history[b]}
        for b in range(N_CORES)
    ]
    res = run_bass_kernel_spmd(nc, in_maps, core_ids=list(range(N_CORES)),
                               trace=trace)
    attn = np.stack([res.results[b]["attn"] for b in range(N_CORES)], axis=0)
    return attn.astype(np.float32), res


def kernel(out_state, history):
    try:
        attn, _ = _run(out_state, history)
    except Exception:
        # one retry, e.g. if a previous process left a core wedged
        attn, _ = _run(out_state, history)
    return attn


# revision 65
# speedup vs baseline: 1.0043x; 1.0043x over previous
"""Trainium2 Bass kernel for batched tanh-query attention.

Per-batch computation (B=8, one batch per NeuronCore, pure data parallel):
    q = tanh(out_state)            [Q, H]    Q=K=2048, H=128
    S = q @ history.T              [Q, K]
    P = softmax(S, axis=K)
    attn = P @ history             [Q, H]

Flash-style in the transposed orientation S_T[k, q] (no transpose of P
needed for the second matmul). Queries processed in 4 quarters of 512
columns; within a quarter a software pipeline at PAIR granularity
(2 k-tiles = 1024 score columns) keeps the Activation engine (the
bottleneck: exp at 1 elem/lane/cycle) saturated:
    PE : MM1 pair p (2x512-col) | MM2 pair p-1 (2 accumulating matmuls)
    ACT: exp over st[128,1024] f32 PSUM -> ex bf16 SBUF
    DVE: binary tree of bf16 adds for the softmax denominator, epilogue
    Pool: f32->bf16 conversion of history chunks
The denominator column d[q] per 128-q tile comes from a tiny matmul
tfin_tile.T @ ones -> [128,1] PSUM, avoiding row-transposes.
PSUM budget (8 banks): st x2 (4) + acc x2 (2) + dcol (1) + tsp (1).
"""

import os
import sys

os.environ.setdefault("NEURON_RT_RESET_CORES", "1")
for _p in ("/opt/trn_rl_repo", "/opt/trn_rl_repo/concourse"):
    if _p not in sys.path:
        sys.path.insert(0, _p)

import numpy as np

N_CORES = 8
SEQ = 2048
H = 128
P = 128
T = SEQ // P          # 16 seq tiles
NQ = 4                # query quarters
QW = SEQ // NQ        # 512
QTPQ = QW // P        # 4 q-tiles per quarter
NPAIR = T // 2        # 8 kb-pairs per quarter

_CACHE = {}


def _build():
    from concourse import bacc, bass, masks, mybir, tile

    f32 = mybir.dt.float32
    bf16 = mybir.dt.bfloat16
    AF = mybir.ActivationFunctionType

    nc = bacc.Bacc("TRN2", target_bir_lowering=False, debug=False,
                   num_devices=N_CORES)
    os_d = nc.dram_tensor("out_state", (SEQ, H), f32, kind="ExternalInput")
    h_d = nc.dram_tensor("history", (SEQ, H), f32, kind="ExternalInput")
    a_d = nc.dram_tensor("attn", (SEQ, H), f32, kind="ExternalOutput")

    with tile.TileContext(nc) as tc:
        with (
            tc.tile_pool(name="const", bufs=1) as constp,
            tc.tile_pool(name="big", bufs=1) as bigp,
            tc.tile_pool(name="ex", bufs=4) as expool,
            tc.tile_pool(name="tree", bufs=2) as treep,
            tc.tile_pool(name="work", bufs=2) as workp,
            tc.tile_pool(name="rcp", bufs=4) as rcp,
            tc.tile_pool(name="psst", bufs=2, space=bass.MemorySpace.PSUM) as psst,
            tc.tile_pool(name="pacc", bufs=2, space=bass.MemorySpace.PSUM) as pacc,
            tc.tile_pool(name="psdc", bufs=1, space=bass.MemorySpace.PSUM) as psdc,
            tc.tile_pool(name="pstp", bufs=1, space=bass.MemorySpace.PSUM) as pstp,
        ):
            # ---- input DMAs first so transfers overlap const setup ----
            os_f = bigp.tile([P, T, H], f32, tag="osf")
            hn_f = bigp.tile([P, T, H], f32, tag="hnf")
            os_v = os_d[:].rearrange("(t p) h -> p t h", p=P)
            hn_v = h_d[:].rearrange("(t p) h -> p t h", p=P)
            a_v = a_d[:].rearrange("(t p) h -> p t h", p=P)
            # os chunks first on sync (so ALL tanh work lands inside the
            # initial DMA-wait window); hn chunk 0 in parallel from the
            # scalar queue (idle until its DMA lands anyway)
            nc.sync.dma_start(os_f[:, 0:4, :], os_v[:, 0:4, :])
            nc.scalar.dma_start(hn_f[:, 0:4, :], hn_v[:, 0:4, :])
            nc.sync.dma_start(os_f[:, 4:8, :], os_v[:, 4:8, :])
            nc.sync.dma_start(os_f[:, 8:16, :], os_v[:, 8:16, :])
            nc.sync.dma_start(hn_f[:, 4:8, :], hn_v[:, 4:8, :])
            nc.sync.dma_start(hn_f[:, 8:12, :], hn_v[:, 8:12, :])
            nc.sync.dma_start(hn_f[:, 12:16, :], hn_v[:, 12:16, :])

            id_bf = constp.tile([P, P], bf16, tag="idb")
            masks.make_identity(nc, id_bf[:])
            ones_bf = constp.tile([P, 1], bf16, tag="ones")
            nc.vector.memset(ones_bf[:], 1.0)

            # persistent bf16 operands
            hn = bigp.tile([P, T, P], bf16, tag="hn")    # [k_in, t, h] natural
            ht = bigp.tile([P, T, P], bf16, tag="ht")    # [h, t, k_in]
            qT = bigp.tile([P, T, P], bf16, tag="qT")    # [h, t, q_in]
            q_nat = bigp.tile([P, T, H], bf16, tag="qnat")
            ot_all = bigp.tile([P, T, H], f32, tag="ot")  # output staging

            # ---- prologue compute ----
            # history f32 -> bf16 on DVE (idle until the first tree add;
            # Pool's software CAST is ~3x slower than DVE here)
            nc.vector.tensor_copy(hn[:, 0:4, :], hn_f[:, 0:4, :])
            nc.scalar.activation(q_nat[:, 0:4, :], os_f[:, 0:4, :], AF.Tanh)
            # history converts are split between DVE (fast, used for the
            # earliest-needed pairs) and Pool (slow but otherwise idle)
            nc.gpsimd.tensor_copy(hn[:, 12:14, :], hn_f[:, 12:14, :])
            nc.gpsimd.tensor_copy(hn[:, 14:16, :], hn_f[:, 14:16, :])

            # PE warm-up: the tensor engine ramps its clock only after ~3us
            # of continuous work; burn the DMA-wait window with dummy
            # transposes so the first real matmuls run at full speed
            wrm = pstp.tile([P, 8, P], bf16, tag="tsp", name="warm")
            for i in range(32):
                nc.tensor.transpose(wrm[:, i % 8, :], id_bf[:], id_bf[:])
            wrm_keep = constp.tile([P, 1], bf16, tag="wk")
            nc.vector.tensor_copy(wrm_keep[:], wrm[:, 0, 0:1])

            # one batch of PE transposes through the single tsp PSUM bank,
            # then one DVE copy out per destination range
            def tp_batch(jobs):
                # jobs: list of (dst_tile, dst_t0, src_tile, src_t0, n)
                ntot = sum(j[4] for j in jobs)
                tsp = pstp.tile([P, 8, P], bf16, tag="tsp", name="tsp")
                s = 0
                for dst, dt0, src, st0, n in jobs:
                    for i in range(n):
                        nc.tensor.transpose(tsp[:, s + i, :],
                                            src[:, st0 + i, :], id_bf[:])
                    s += n
                s = 0
                for dst, dt0, src, st0, n in jobs:
                    nc.vector.tensor_copy(dst[:, dt0:dt0 + n, :],
                                          tsp[:, s:s + n, :])
                    s += n

            tp_batch([(qT, 0, q_nat, 0, 4), (ht, 0, hn, 0, 2)])
            tp_batch([(ht, 2, hn, 2, 2)])

            # ---- epilogue helper: one quarter's outputs ----
            # returns closures to be emitted at chosen pipeline slots
            def make_epilogue(q, acc, dsrcs):
                st8 = {}

                def pe_dcols():
                    # one-column accumulating matmuls into one PSUM tile,
                    # then a single reciprocal over all 4 columns
                    dc = psdc.tile([P, QTPQ], f32, tag="dc", name=f"dc{q}")
                    for t in range(QTPQ):
                        for i, src in enumerate(dsrcs):
                            nc.tensor.matmul(dc[:, t:t + 1],
                                             src[:, P * t:P * (t + 1)],
                                             ones_bf[:], start=(i == 0),
                                             stop=(i == len(dsrcs) - 1))
                    rc = rcp.tile([P, QTPQ], f32, tag="rc", name=f"rc{q}")
                    nc.vector.reciprocal(rc[:], dc[:])
                    st8["rc"] = rc

                def dve_copy():
                    aT = workp.tile([P, QW], bf16, tag="at", name=f"aT{q}")
                    if q == NQ - 1:
                        # split between DVE and the now-idle ACT engine so
                        # the tail's transposes start half a copy earlier
                        nc.vector.tensor_copy(aT[:, 0:QW // 2],
                                              acc[:, 0:QW // 2])
                        nc.scalar.activation(aT[:, QW // 2:],
                                             acc[:, QW // 2:], AF.Copy)
                    else:
                        nc.vector.tensor_copy(aT[:], acc[:])
                    st8["aT"] = aT

                def pe_transposes():
                    tsp = pstp.tile([P, 8, P], bf16, tag="tsp", name=f"ep{q}")
                    for t in range(QTPQ):
                        nc.tensor.transpose(tsp[:, t, :],
                                            st8["aT"][:, P * t:P * (t + 1)],
                                            id_bf[:])
                    st8["tsp"] = tsp

                def dve_muls():
                    for t in range(QTPQ):
                        if q == NQ - 1 and t >= 2:
                            # ACT is idle after the last exp; Copy-with-scale
                            # halves the tail's serial mul chain
                            nc.scalar.activation(
                                ot_all[:, QTPQ * q + t, :],
                                st8["tsp"][:, t, :], AF.Copy,
                                scale=st8["rc"][:, t:t + 1])
                        else:
                            nc.vector.tensor_scalar_mul(
                                ot_all[:, QTPQ * q + t, :],
                                st8["tsp"][:, t, :], st8["rc"][:, t:t + 1])

                def dma_out():
                    if q == NQ - 1:
                        # split so the first half's transfer overlaps the
                        # second half's muls
                        nc.sync.dma_start(a_v[:, QTPQ * q:QTPQ * q + 2, :],
                                          ot_all[:, QTPQ * q:QTPQ * q + 2, :])
                        nc.sync.dma_start(
                            a_v[:, QTPQ * q + 2:QTPQ * (q + 1), :],
                            ot_all[:, QTPQ * q + 2:QTPQ * (q + 1), :])
                    else:
                        nc.sync.dma_start(a_v[:, QTPQ * q:QTPQ * (q + 1), :],
                                          ot_all[:, QTPQ * q:QTPQ * (q + 1), :])

                return pe_dcols, dve_copy, pe_transposes, dve_muls, dma_out

            # ---- main pipeline ----
            accs = [None] * NQ
            ex_tiles = [[None] * NPAIR for _ in range(NQ)]
            epi = [None] * NQ

            def emit_mm1(q, p):
                st = psst.tile([P, 2 * QW], f32, tag="st", name=f"st{q}{p}")
                rhs = qT[:, QTPQ * q:QTPQ * (q + 1), :]
                nc.tensor.matmul(st[:, 0:QW], ht[:, 2 * p, :], rhs,
                                 start=True, stop=True)
                nc.tensor.matmul(st[:, QW:], ht[:, 2 * p + 1, :], rhs,
                                 start=True, stop=True)
                return st

            def emit_exp(q, p, st):
                ex = expool.tile([P, 2 * QW], bf16, tag="ex", name=f"ex{q}{p}")
                if q == NQ - 1 and p == NPAIR - 1:
                    # split the very last exp so the final MM2 (and with it
                    # the whole output tail) starts half an exp earlier
                    nc.scalar.activation(ex[:, 0:QW], st[:, 0:QW], AF.Exp)
                    nc.scalar.activation(ex[:, QW:], st[:, QW:], AF.Exp)
                else:
                    nc.scalar.activation(ex[:], st[:], AF.Exp)
                ex_tiles[q][p] = ex

            def emit_mm2(q, p):
                if accs[q] is None:
                    accs[q] = pacc.tile([P, QW], f32, tag="acc",
                                        name=f"acc{q}")
                for kb in (2 * p, 2 * p + 1):
                    nc.tensor.matmul(
                        accs[q][:], hn[:, kb, :],
                        ex_tiles[q][p][:, QW * (kb % 2):QW * (kb % 2 + 1)],
                        start=(kb == 0), stop=(kb == T - 1))

            # denominator tree state per quarter
            l1s = [[None, None, None, None] for _ in range(NQ)]
            l2s = [[None, None] for _ in range(NQ)]
            tfins = [None] * NQ

            def emit_tree(q, p):
                # called at odd p: add pair (p-1, p); fold levels when ready
                i = p // 2
                t1 = treep.tile([P, 2 * QW], bf16, tag="l1", name=f"l1_{q}{i}")
                nc.vector.tensor_add(t1[:], ex_tiles[q][p - 1][:],
                                     ex_tiles[q][p][:])
                l1s[q][i] = t1
                if p in (3, 7):
                    j = p // 4
                    t2 = treep.tile([P, 2 * QW], bf16, tag="l2",
                                    name=f"l2_{q}{j}")
                    nc.vector.tensor_add(t2[:], l1s[q][2 * j][:],
                                         l1s[q][2 * j + 1][:])
                    l2s[q][j] = t2
                if p == 7:
                    t3 = treep.tile([P, 2 * QW], bf16, tag="l3", name=f"l3_{q}")
                    nc.vector.tensor_add(t3[:], l2s[q][0][:], l2s[q][1][:])
                    tf = treep.tile([P, QW], bf16, tag="tf", name=f"tf{q}")
                    nc.vector.tensor_add(tf[:], t3[:, 0:QW], t3[:, QW:])
                    tfins[q] = tf

            def emit_tree_last(q, p):
                # last quarter: running sum so only the final pair's fold
                # sits on the critical path after the last exp
                i = p // 2
                if p in (1, 3, 5):
                    t1 = treep.tile([P, 2 * QW], bf16, tag="l1",
                                    name=f"l1_{q}{i}")
                    nc.vector.tensor_add(t1[:], ex_tiles[q][p - 1][:],
                                         ex_tiles[q][p][:])
                    l1s[q][i] = t1
                if p == 3:
                    t2 = treep.tile([P, 2 * QW], bf16, tag="l2",
                                    name=f"l2_{q}0")
                    nc.vector.tensor_add(t2[:], l1s[q][0][:], l1s[q][1][:])
                    l2s[q][0] = t2
                if p == 5:
                    t2 = treep.tile([P, 2 * QW], bf16, tag="l2",
                                    name=f"l2_{q}1")
                    nc.vector.tensor_add(t2[:], l2s[q][0][:], l1s[q][2][:])
                    l2s[q][1] = t2
                if p == 6:
                    # fold in pair 6 and collapse to one 512-wide tile
                    t3 = treep.tile([P, 2 * QW], bf16, tag="l3", name=f"l3_{q}")
                    nc.vector.tensor_add(t3[:], l2s[q][1][:],
                                         ex_tiles[q][6][:])
                    tf = treep.tile([P, QW], bf16, tag="tf", name=f"tf{q}")
                    nc.vector.tensor_add(tf[:], t3[:, 0:QW], t3[:, QW:])
                    tfins[q] = tf

            def fold_last_pair(q):
                # fold the final pair's exp tile to 512 wide on DVE; the
                # epilogue dcols then accumulate tfin + this fold
                f7 = treep.tile([P, QW], bf16, tag="f7", name=f"f7_{q}")
                nc.vector.tensor_add(f7[:], ex_tiles[q][NPAIR - 1][:, 0:QW],
                                     ex_tiles[q][NPAIR - 1][:, QW:])
                return f7

            for q in range(NQ):
                last = q == NQ - 1
                # MM1 leads the exp stream by two pairs so each exp's input
                # is complete well before the previous exp retires; the
                # leading MM1 is emitted LAST in each slot so batch/epilogue
                # PE work fills the WAR wait on the st buffer
                sts = {0: emit_mm1(q, 0), 1: emit_mm1(q, 1)}
                for p in range(NPAIR):
                    emit_exp(q, p, sts.pop(p))
                    if p > 0:
                        emit_mm2(q, p - 1)
                    if last:
                        if p % 2 == 1 or p == 6:
                            emit_tree_last(q, p)
                    elif p % 2 == 1:
                        emit_tree(q, p)
                    if q == 0 and p in (2, 4, 6):
                        # tanh for later quarters, filling ACT stalls while
                        # quarter 0 waits on the history convert chain
                        j0 = 2 * p
                        nc.scalar.activation(q_nat[:, j0:j0 + 4, :],
                                             os_f[:, j0:j0 + 4, :], AF.Tanh)
                    if q == 0:
                        # DVE casts for the earliest-needed history pairs,
                        # then ht transposes one pair ahead of their consumer
                        if p <= 3:
                            j0 = 4 + 2 * p
                            nc.vector.tensor_copy(hn[:, j0:j0 + 2, :],
                                                  hn_f[:, j0:j0 + 2, :])
                        if p <= 5:
                            t0 = 2 * p + 4
                            tp_batch([(ht, t0, hn, t0, 2)])
                    if q >= 1 and epi[q - 1] is not None:
                        pe_dcols, dve_copy, pe_trans, dve_muls, dma_out = \
                            epi[q - 1]
                        if p == 0:
                            # slot 0 has no MM2, so the epilogue's PE burst
                            # (d-column matmuls) lands in spare PE time
                            # instead of displacing the leading MM1s
                            dve_copy()
                            pe_dcols()
                        elif p == 1:
                            pe_trans()
                        elif p == 2:
                            dve_muls()
                        elif p == 3:
                            dma_out()
                            epi[q - 1] = None
                    if p == 7 and q < NQ - 1:
                        tp_batch([(qT, QTPQ * (q + 1), q_nat,
                                   QTPQ * (q + 1), 4)])
                    if p + 2 < NPAIR:
                        sts[p + 2] = emit_mm1(q, p + 2)
                # close the quarter
                emit_mm2(q, NPAIR - 1)
                if last:
                    f7 = fold_last_pair(q)
                    epi[q] = make_epilogue(q, accs[q], [tfins[q], f7])
                else:
                    epi[q] = make_epilogue(q, accs[q], [tfins[q]])

            # final quarter epilogue, emitted tight
            pe_dcols, dve_copy, pe_trans, dve_muls, dma_out = epi[NQ - 1]
            dve_copy()
            pe_dcols()
            pe_trans()
            dve_muls()
            dma_out()

    nc.compile()
    return nc


def _get_nc():
    if "nc" not in _CACHE:
        _CACHE["nc"] = _build()
    return _CACHE["nc"]


def _run(out_state, history, trace=False):
    from concourse.bass_utils import run_bass_kernel_spmd

    nc = _get_nc()
    out_state = np.ascontiguousarray(out_state, dtype=np.float32)
    history = np.ascontiguousarray(history, dtype=np.float32)
    in_maps = [
        {"out_state": out_state[b], "history": history[b]}
        for b in range(N_CORES)
    ]
    res = run_bass_kernel_spmd(nc, in_maps, core_ids=list(range(N_CORES)),
                               trace=trace)
    attn = np.stack([res.results[b]["attn"] for b in range(N_CORES)], axis=0)
    return attn.astype(np.float32), res


def kernel(out_state, history):
    try:
        attn, _ = _run(out_state, history)
    except Exception:
        # one retry, e.g. if a previous process left a core wedged
        attn, _ = _run(out_state, history)
    return attn
